# revision 1
# baseline (speedup 1.0000x reference)
"""FFM (field-aware factorization machine) forward kernel for 8 TRN2 NeuronCores.

y[b] = x[b] @ w_lin + b_lin + sum_{i<j} Wu[i,j] x[b,i] x[b,j]
with Wu = triu(Wmat, 1), Wmat[i,j] = <v[i, field[j]], v[j, field[i]]>.

Strategy (v3, measured ~21.7us/rep vs 31us baseline on this fabric):
  - Host: build Wmat from (v, field_idx) [tiny], symmetrize
    S = (Wu + Wu^T)/2, eigendecompose S = Q diag(lam) Q^T. Shift the
    spectrum by c = -lam_min so mu = lam + c >= 0, fold sqrt(mu) into the
    eigenvectors: Q' = Q diag(sqrt(mu)). Then
      x^T Wu x = sum_n mu_n (x . q_n)^2 - c ||x||^2
    and the -c||x||^2 correction joins the (host-computed) linear part.
    The all-positive weights mean the device reduction needs NO pos/neg
    split: one fused square-accumulate per batch chunk.
  - Device (data-parallel over batch, 8 cores): per 128-sample chunk,
    PE computes z' = x_chunk^T Q' with batch on PSUM partitions and the
    eigen index on the free dim (two bf16 matmuls, contraction 256).
    The per-sample reduction sum_n z'_n^2 then runs along the FREE dim
    on ScalarE/VectorE (see _build_nc docstring). No PE reduce matmuls.
  - x ships as bf16 (halves HBM traffic vs fp32; measured rel err 1.25%
    against the 2e-2 budget), pre-transposed so the contraction dim
    lands on SBUF partitions, in single-DMA-per-slab layout.
  - Measured walls per rep on this hardware: PE 20.5us (64 chunks x 2
    matmuls x (128cy ldweights + 256cy stream) -- weight loads do NOT
    hide behind streams), DMA 16.2us (~270 GB/s effective), reducers
    ~19us balanced. The kernel sits at its PE wall.
"""

import numpy as np

_B, _N = 65536, 256
_NCORES = 8
_BS = _B // _NCORES   # 8192 samples per core
_NCH = _BS // 128     # 64 batch chunks per core
_DCH = 2048           # DMA chunk columns (512 KiB per half-slab)
_NDMA = _BS // _DCH   # 4 DMA iterations
_KPER = _DCH // 128   # 16 chunks per DMA iteration

_compiled_nc = {}


# DMA column schedule: uniform slabs keep the (serial, atomic) DMA transfer
# stream matched to PE consumption — non-uniform slabs stall PE at slab
# boundaries. Slab count is capped by the ~1.2us fixed cost per DMA
# instruction (SP sequencer + HWDGE descriptor generation). Must sum to _BS.
_DMA_SCHED = (1024,) * 8


def _build_nc(reps=1, mode="full", act_chunks=36, xin_bufs=3,
              gsz=2, sched=_DMA_SCHED, dve_mode="stt"):
    """Flipped-eigen FFM kernel.

    Per 128-sample chunk PE computes z' = x^T Q' into PSUM ([128, 256],
    batch on partitions). The per-sample reduce sum_n z'^2 runs along the
    free dim. PSUM operands can only be read once per DVE instruction
    (walrus NCC_IBVF027) and GpSimd has no PSUM port, so chunks are
    processed in groups of 4 sharing a 4-bank PSUM tile, split two ways:
      - ACT chunks: ScalarE Square(psum)+accum_out (the only engine that
        can square straight out of PSUM).
      - DVE chunks: one VectorE copy moves the group's remaining chunks'
        z to SBUF bf16, then per-chunk tensor_tensor_reduce squares and
        accumulates.
    act_chunks of the 64 chunks go to ScalarE; within each group the
    first n_act are ACT's, the tail is one strided copy.
    """
    from concourse import bacc, mybir, tile

    f32 = mybir.dt.float32
    bf16 = mybir.dt.bfloat16
    Act = mybir.ActivationFunctionType
    Alu = mybir.AluOpType

    assert sum(sched) == _BS
    n_groups = _NCH // gsz
    # Polarized group types (all-ACT or all-DVE, to maximize copy batching)
    # interleaved in time so both engines drain groups concurrently.
    full_a, rem = divmod(act_chunks, gsz)
    is_act = [(g + 1) * full_a // n_groups > g * full_a // n_groups
              for g in range(n_groups)]
    n_act_of = [gsz if a else 0 for a in is_act]
    if rem:
        n_act_of[next(g for g in range(n_groups) if not is_act[g])] = rem

    nc = bacc.Bacc("TRN2", target_bir_lowering=False, debug=False)

    # x^T and Q' with the 256-row contraction dim split into 2 blocks of
    # 128 partitions; [blk, 128, cols] in DRAM, loaded as [128, blk, cols]
    # so each slab (both blocks) is a single DMA instruction.
    xt = nc.dram_tensor("xt", [2, 128, _BS], bf16, kind="ExternalInput").ap()
    qp = nc.dram_tensor("qp", [2, 128, _N], bf16, kind="ExternalInput").ap()
    # y[p, c] = sum_n z'^2 for sample c*128 + p (per core)
    y = nc.dram_tensor("y", [128, _NCH], f32, kind="ExternalOutput").ap()

    max_dch = max(sched)

    with tile.TileContext(nc) as tc:
        with (
            tc.tile_pool(name="const", bufs=1) as cpool,
            tc.tile_pool(name="xin", bufs=xin_bufs) as xpool,
            tc.tile_pool(name="yout", bufs=2) as ypool,
            tc.tile_pool(name="zsb", bufs=3) as zpool,
            tc.tile_pool(name="scr", bufs=4) as spool,
            tc.tile_pool(name="pz", bufs=8 // gsz, space="PSUM") as pzpool,
        ):
            q_sb = cpool.tile([128, 2, _N], bf16)
            nc.sync.dma_start(q_sb[:], qp[:, :, :].transpose([1, 0, 2]))

            # chunk c occupies pz4[:, (c%4)*512 : (c%4)*512+256]; the upper
            # half of each bank is ACT's Square scratch.
            def emit_group(g, pz4, y_sb):
                n_act = n_act_of[g]
                for j in range(n_act):
                    c = g * gsz + j
                    nc.scalar.activation(
                        pz4[:, j * 512 + 256:j * 512 + 512],
                        pz4[:, j * 512:j * 512 + 256],
                        Act.Square, accum_out=y_sb[:, c:c + 1])
                k = gsz - n_act
                if k == 0:
                    return
                z_sb = zpool.tile([128, gsz, 256], bf16, tag="z")
                if dve_mode == "copy2d":
                    for i in range(k):
                        nc.vector.tensor_copy(
                            z_sb[:, i, :],
                            pz4[:, (n_act + i) * 512:(n_act + i) * 512 + 256])
                else:
                    nc.vector.tensor_copy(
                        z_sb[:, 0:k, :],
                        pz4[:].rearrange("p (k c) -> p k c", k=gsz)
                           [:, n_act:gsz, 0:256])
                for i in range(k):
                    c = g * gsz + n_act + i
                    zj = z_sb[:, i, :]
                    ycol = y_sb[:, c:c + 1]
                    if dve_mode == "actred":
                        scr = spool.tile([128, 256], bf16, tag="s")
                        nc.scalar.activation(scr[:], zj, Act.Square,
                                             accum_out=ycol)
                    elif dve_mode == "stt":
                        scr = spool.tile([128, 256], bf16, tag="s")
                        nc.vector.scalar_tensor_tensor(
                            out=scr[:], in0=zj, scalar=1.0, in1=zj,
                            op0=Alu.mult, op1=Alu.mult, accum_out=ycol)
                    elif dve_mode == "ttr_f32":
                        scr = spool.tile([128, 256], f32, tag="sf")
                        nc.vector.tensor_tensor_reduce(
                            out=scr[:], in0=zj, in1=zj, scale=1.0,
                            scalar=0.0, op0=Alu.mult, op1=Alu.add,
                            accum_out=ycol)
                    else:
                        scr = spool.tile([128, 256], bf16, tag="s")
                        nc.vector.tensor_tensor_reduce(
                            out=scr[:], in0=zj, in1=zj, scale=1.0,
                            scalar=0.0, op0=Alu.mult, op1=Alu.add,
                            accum_out=ycol)

            for _rep in range(reps):
                y_sb = ypool.tile([128, _NCH], f32, tag="y")
                c = 0
                off = 0
                for dch in sched:
                    x_sb = xpool.tile([128, 2, max_dch], bf16, tag="x")
                    if mode != "noxdma":
                        nc.sync.dma_start(
                            x_sb[:, :, 0:dch],
                            xt[:, :, off:off + dch].transpose([1, 0, 2]))
                    off += dch
                    if mode == "dmaonly":
                        continue
                    for k in range(dch // 128):
                        j = c % gsz
                        if j == 0:
                            pz4 = pzpool.tile([128, gsz * 512], f32,
                                              tag="pz")
                        if mode == "wide":
                            # perf probe: one 512-wide matmul per chunk
                            # (junk values; measures ld-vs-stream overlap)
                            nc.tensor.matmul(pz4[:, j * 512:(j + 1) * 512],
                                             x_sb[:, 0,
                                                  k * 128:(k + 1) * 128],
                                             q_sb[:].rearrange(
                                                 "p a n -> p (a n)"),
                                             start=True, stop=True)
                            c += 1
                            continue
                        nc.tensor.matmul(pz4[:, j * 512:j * 512 + 256],
                                         x_sb[:, 0, k * 128:(k + 1) * 128],
                                         q_sb[:, 0, :], start=True, stop=False)
                        nc.tensor.matmul(pz4[:, j * 512:j * 512 + 256],
                                         x_sb[:, 1, k * 128:(k + 1) * 128],
                                         q_sb[:, 1, :], start=False, stop=True)
                        c += 1
                        if mode == "nored":
                            continue
                        if j == gsz - 1:
                            emit_group(c // gsz - 1, pz4, y_sb)
                if mode == "full":
                    nc.sync.dma_start(y[:, :], y_sb[:])

    nc.compile()
    return nc


def _get_nc(reps=1, **kw):
    key = (reps,) + tuple(sorted(kw.items()))
    if key not in _compiled_nc:
        _compiled_nc[key] = _build_nc(reps, **kw)
    return _compiled_nc[key]


def _to_bf16(a):
    import ml_dtypes

    return np.ascontiguousarray(a).astype(ml_dtypes.bfloat16)


def _host_prep(x, w_lin, b_lin, v, field_idx):
    """Host-side tiny-param preprocessing + sharding. Returns (in_maps, lin)."""
    x = np.asarray(x, dtype=np.float32)
    w_lin = np.asarray(w_lin, dtype=np.float64)
    b_lin = np.asarray(b_lin, dtype=np.float64)
    v = np.asarray(v, dtype=np.float64)
    field_idx = np.asarray(field_idx, dtype=np.int64)

    # Wmat[i, j] = <v[i, field[j]], v[j, field[i]]>
    A = v[:, field_idx, :]                       # [N, N, K]
    Wmat = np.einsum('ijk,jik->ij', A, A)        # [N, N]
    Wu = np.triu(Wmat, 1)
    S = (Wu + Wu.T) * 0.5
    lam, Q = np.linalg.eigh(S)                   # ascending eigenvalues
    c = max(0.0, -lam[0])
    mu = np.clip(lam + c, 0.0, None)
    # [N, N] column-scaled, contraction split into 2 blocks of 128 rows
    Qp = _to_bf16(Q * np.sqrt(mu)[None, :]).reshape(2, 128, _N)

    # x transposed + sharded along batch, bf16, [2, 128, BS] per core
    x64 = x.astype(np.float64)
    xts = x.reshape(_NCORES, _BS, _N).transpose(0, 2, 1)  # [8, N, BS]
    xts = _to_bf16(xts).reshape(_NCORES, 2, 128, _BS)

    in_maps = [{"xt": xts[i], "qp": Qp} for i in range(_NCORES)]
    # linear part and the -c||x||^2 spectrum-shift correction, both host-side
    lin = x64 @ w_lin + b_lin[0] - c * np.einsum('bi,bi->b', x64, x64)
    return in_maps, lin


def _run_device(in_maps, trace=False, reps=1):
    from concourse.bass_utils import run_bass_kernel_spmd

    nc = _get_nc(reps)
    res = run_bass_kernel_spmd(
        nc, in_maps, core_ids=list(range(_NCORES)), trace=trace
    )
    # y[p, c] holds sample c*128+p -> transpose to batch order
    yq = np.concatenate(
        [np.asarray(res.results[i]["y"], dtype=np.float64).T.reshape(-1)
         for i in range(_NCORES)]
    )
    return yq, res


def kernel(x, w_lin, b_lin, v, field_idx):
    in_maps, lin = _host_prep(x, w_lin, b_lin, v, field_idx)
    yq, _ = _run_device(in_maps, trace=False)
    return (lin + yq).astype(np.float32)[:, None]

